# revision 1
# baseline (speedup 1.0000x reference)
"""LSTM (T=4096, B=2048, I=1, H=4) + linear head, on 8 trn2 NeuronCores.

Strategy
--------
The scan has exponential state washout (forget gates ~sigmoid(small)), so the
time axis is split into chunks: each chunk is computed from zero state with a
WARM-step warmup whose outputs are discarded (init error ~2e-8 at WARM=64 in
fp64, far below the fp16 data-path noise; verified numerically).  Every core
runs STREAMS independent chunks interleaved instruction-by-instruction, each
covering all 2048 sequences, so per-step engine ops stay wide and the serial
dependency chains of the streams hide each other's latency.

Per-core layout: batch = 8 slices x 256 columns.  Gate partition index
(j, s) = 4 hidden units x 8 slices = 32 rows per gate block.  One K=48,
M=128 block-diagonal matmul per step computes all gate pre-activations:
  rows k = (c, s), c in {h0..h3, ones, x};  cols m = gate blocks [i|f|o|g],
  each col (blk, j', s') holding w_hh[row(blk,j'), c] * delta_ss'.  The ones
  row carries the biases, the x row carries w_ih, and 0.5 is baked into the
  i,f,o columns so one Tanh covers all gates (sigma = 0.5*tanh(a/2)+0.5).
All elementwise work runs on same-base SBUF fp16 tiles (DVE 2x/4x modes, no
PSUM access bubbles); the single-source ops (tanh, affine) relocate blocks
across partition bases so every tensor_tensor pair is base-aligned, which
also satisfies the BIR verifier rule that two SBUF inputs share a base.
The state ring (h rows 0:32, ones 32:40, x 40:48) doubles as the h history;
a small second matmul (w_fc block-diagonal, M=8) computes the output
projection every FCG steps, rotating PSUM partition bases so the PSUM->SBUF
copy amortizes over FCG*FCS steps.

Written in raw Bass (explicit per-engine streams + counting semaphores):
the Tile scheduler's multi-wait instructions are rejected by this walrus
build ("Too many sync wait commands").
"""

import numpy as np

T, B, I, H = 4096, 2048, 1, 4
NCORES = 8
STREAMS = 2
FP16 = True
WARM = 64
RING = 64
FCG = 4              # steps per output-projection group
FCS = 4              # groups per PSUM->SBUF copy (rotating partition bases)
SLICES = 8
COLS = B // SLICES   # 256
CHUNK = T // (NCORES * STREAMS)
NT = CHUNK + WARM
XCH = 32             # x-prefetch chunk (ring slots per DMA)
assert NT % RING == 0 and RING % FCG == 0 and RING % XCH == 0
NGROUPS = NT // FCG + 1
YPAD = NGROUPS * FCG
SUPER = FCG * FCS    # steps per stage copy

GATE_SCALE = (0.5, 0.5, 0.5, 1.0)   # blocks [f, o, i, g]
REF_ROW = (4, 12, 0, 8)             # block -> first row in reference order


def _prep_weights(w_ih, w_hh, b_ih, b_hh, w_fc, b_fc):
    dt = np.float16 if FP16 else np.float32
    bias = (b_ih + b_hh).astype(np.float64)
    wblk = np.zeros((48, 128), np.float64)
    wfc = np.zeros((48, 8), np.float64)
    for s in range(SLICES):
        for blk in range(4):
            sc = GATE_SCALE[blk]
            for j in range(4):
                row = REF_ROW[blk] + j
                m = blk * 32 + j * 8 + s
                for c in range(4):
                    wblk[c * 8 + s, m] = w_hh[row, c] * sc
                wblk[32 + s, m] = bias[row] * sc
                wblk[40 + s, m] = w_ih[row, 0] * sc
        for c in range(4):
            wfc[c * 8 + s, s] = w_fc[0, c]
        wfc[32 + s, s] = b_fc[0]
    return wblk.astype(dt), wfc.astype(dt)


def _build_program():
    from contextlib import ExitStack
    import concourse.bass as bass
    from concourse import mybir

    fp32 = mybir.dt.float32
    fpw = mybir.dt.float16 if FP16 else mybir.dt.float32
    TT = mybir.AluOpType
    Act = mybir.ActivationFunctionType

    nc = bass.Bass("TRN2", target_bir_lowering=False, debug=False,
                   num_devices=NCORES)
    xcd = nc.dram_tensor("xc", [STREAMS, NT, B], fpw, kind="ExternalInput")
    wblkd = nc.dram_tensor("wblk", [48, 128], fpw, kind="ExternalInput")
    wfcd = nc.dram_tensor("wfc", [48, 8], fpw, kind="ExternalInput")
    ycd = nc.dram_tensor("yc", [STREAMS, YPAD, B], fp32, kind="ExternalOutput")

    NCHUNK = NT // XCH
    NSUPER = NT // SUPER
    GFIN = NT // FCG

    with ExitStack() as ctx:
        ec = ctx.enter_context
        block = ec(nc.Block())
        sem = {}
        for st in range(STREAMS):
            for name in ("pe", "pe2", "act", "dvec", "dveh", "copy",
                         "xsem", "wsem", "init", "osem0", "osem1"):
                sem[st, name] = ec(nc.semaphore(f"{name}{st}"))
        # SBUF state/work tiles (all fp16).  Base partitions chosen so every
        # tensor_tensor input pair shares a base:
        #   tgS:  tanh(a/2) blocks [f|o|i] rows 0:96, tanh(ag) rows 96:128
        #   sigX: sig_f rows 0:32, sig_o rows 32:64, sig_i rows 96:128
        #   cF:   c rows 0:32;  tctF: tanh(c) rows 32:64
        # (engine APs spanning >32 partitions must start at partition 0)
        sring, tgS, sigX, cF, tctF, igb, fcb, stage = ({} for _ in range(8))
        for st in range(STREAMS):
            sring[st] = ec(nc.sbuf_tensor(f"sring{st}", [48, RING, COLS], fpw))
            tgS[st] = ec(nc.sbuf_tensor(f"tgS{st}", [128, COLS], fpw))
            sigX[st] = ec(nc.sbuf_tensor(f"sigX{st}", [128, COLS], fpw))
            cF[st] = ec(nc.sbuf_tensor(f"cF{st}", [32, COLS], fpw))
            tctF[st] = ec(nc.sbuf_tensor(f"tctF{st}", [64, COLS], fpw))
            igb[st] = ec(nc.sbuf_tensor(f"igb{st}", [32, COLS], fpw))
            fcb[st] = ec(nc.sbuf_tensor(f"fcb{st}", [32, COLS], fpw))
            stage[st] = [ec(nc.sbuf_tensor(f"stage{st}_{i}", [104, FCG * COLS], fp32))
                         for i in range(2)]
        wblk = ec(nc.sbuf_tensor("wblk_sb", [48, 128], fpw))
        wfc = ec(nc.sbuf_tensor("wfc_sb", [48, 8], fpw))
        gates, fcps = {}, {}
        for st in range(STREAMS):
            gates[st] = ec(nc.psum_tensor(f"gates{st}", [128, COLS], fp32))
            fcps[st] = ec(nc.psum_tensor(f"fcps{st}", [104, FCG * COLS], fp32))

        xv = {st: xcd.ap()[st].rearrange("t (s c) -> s t c", s=SLICES)
              for st in range(STREAMS)}

        @block.sync
        def _(sp):
            sp.dma_start(wblk.ap(), wblkd.ap()).then_inc(sem[0, "wsem"], 16)
            sp.dma_start(wfc.ap(), wfcd.ap()).then_inc(sem[0, "wsem"], 16)
            for k in range(NCHUNK):
                for st in range(STREAMS):
                    if k >= 2:
                        sp.wait_ge(sem[st, "pe"], XCH * (k - 1))
                    slot = (k * XCH) % RING
                    sp.dma_start(
                        sring[st].ap()[40:48, slot:slot + XCH, :],
                        xv[st][:, k * XCH:(k + 1) * XCH, :],
                    ).then_inc(sem[st, "xsem"], 16)

        def _fc_group(pe, st, g):
            q = g % FCS
            if g >= FCS:
                pe.wait_ge(sem[st, "copy"], g // FCS)
            ins = None
            for pair in range(FCG // 2):
                slot0 = (g * FCG) % RING + pair * 2
                ins = pe.matmul(
                    fcps[st].ap()[32 * q:32 * q + 8, pair * 512:pair * 512 + 512],
                    wfc.ap(),
                    sring[st].ap()[:, slot0:slot0 + 2, :],
                    start=True, stop=True, tile_position=(0, 32 * q),
                )
            ins.then_inc(sem[st, "pe2"], 1)

        @block.tensor
        def _(pe):
            pe.wait_ge(sem[0, "wsem"], 32)
            for st in range(STREAMS):
                pe.wait_ge(sem[st, "init"], 1)
            for t in range(NT):
                for st in range(STREAMS):
                    if t % XCH == 0:
                        pe.wait_ge(sem[st, "xsem"], 16 * (t // XCH + 1))
                    if t > 0:
                        pe.wait_ge(sem[st, "dveh"], t)
                    pe.matmul(gates[st].ap(), wblk.ap(),
                              sring[st].ap()[:, t % RING, :],
                              start=True, stop=True).then_inc(sem[st, "pe"], 1)
                    if t % FCG == FCG - 1:
                        _fc_group(pe, st, (t - (FCG - 1)) // FCG)
            for st in range(STREAMS):
                pe.wait_ge(sem[st, "dveh"], NT)
                _fc_group(pe, st, GFIN)

        @block.scalar
        def _(act):
            for t in range(NT):
                for st in range(STREAMS):
                    act.wait_ge(sem[st, "pe"], t + 1)
                    act.activation(tgS[st].ap(), gates[st].ap(),
                                   Act.Tanh).then_inc(sem[st, "act"], 1)
                for st in range(STREAMS):
                    act.wait_ge(sem[st, "dvec"], t + 1)
                    act.activation(tctF[st].ap()[32:64], cF[st].ap(),
                                   Act.Tanh).then_inc(sem[st, "act"], 1)

        def _stage_copy(dve, st, s, full):
            ngr = FCS if full else 1
            dve.wait_ge(sem[st, "pe2"], FCS * s + ngr)
            if s >= 2:
                dve.wait_ge(sem[st, "osem0" if s % 2 == 0 else "osem1"],
                            64 * (s // 2))
            nrows = 32 * (ngr - 1) + 8
            dve.tensor_copy(stage[st][s % 2].ap()[0:nrows, :],
                            fcps[st].ap()[0:nrows, :]
                            ).then_inc(sem[st, "copy"], 1)

        @block.vector
        def _(dve):
            for st in range(STREAMS):
                dve.memset(sring[st].ap()[0:32, 0, :], 0.0)
                dve.memset(sring[st].ap()[32:40, :, :], 1.0)
                dve.memset(cF[st].ap(), 0.0)
                dve.memset(fcps[st].ap(), 0.0).then_inc(sem[st, "init"], 1)
            for t in range(NT):
                for st in range(STREAMS):
                    dve.wait_ge(sem[st, "act"], 2 * t + 1)
                    # sig_f, sig_o at rows 0:64 (one op from base 0)
                    dve.tensor_scalar(sigX[st].ap()[0:64], tgS[st].ap()[0:64],
                                      0.5, 0.5, TT.mult, TT.add)
                    # sig_i -> rows 96:128 (base-aligns with tanh(g))
                    dve.tensor_scalar(sigX[st].ap()[96:128], tgS[st].ap()[64:96],
                                      0.5, 0.5, TT.mult, TT.add)
                    dve.tensor_tensor(igb[st].ap(), sigX[st].ap()[96:128],
                                      tgS[st].ap()[96:128], TT.mult)
                    dve.tensor_tensor(fcb[st].ap(), sigX[st].ap()[0:32],
                                      cF[st].ap(), TT.mult)
                    dve.tensor_tensor(cF[st].ap(), igb[st].ap(),
                                      fcb[st].ap(), TT.add
                                      ).then_inc(sem[st, "dvec"], 1)
                for st in range(STREAMS):
                    dve.wait_ge(sem[st, "act"], 2 * t + 2)
                    dve.tensor_tensor(sring[st].ap()[0:32, (t + 1) % RING, :],
                                      sigX[st].ap()[32:64], tctF[st].ap()[32:64],
                                      TT.mult).then_inc(sem[st, "dveh"], 1)
                    if t % SUPER == SUPER - 1:
                        _stage_copy(dve, st, t // SUPER, full=True)
            for st in range(STREAMS):
                _stage_copy(dve, st, NSUPER, full=False)

        @block.gpsimd
        def _(gp):
            for s in range(NSUPER):
                for st in range(STREAMS):
                    gp.wait_ge(sem[st, "copy"], s + 1)
                    for q in range(FCS):
                        grow = (s * FCS + q) * FCG
                        gp.dma_start(
                            ycd.ap()[st, grow:grow + FCG].rearrange(
                                "t (s c) -> s t c", s=SLICES),
                            stage[st][s % 2].ap()[32 * q:32 * q + 8, :].rearrange(
                                "s (t c) -> s t c", t=FCG),
                        ).then_inc(sem[st, "osem0" if s % 2 == 0 else "osem1"], 16)
            for st in range(STREAMS):
                gp.wait_ge(sem[st, "copy"], NSUPER + 1)
                grow = GFIN * FCG
                gp.dma_start(
                    ycd.ap()[st, grow:grow + FCG].rearrange(
                        "t (s c) -> s t c", s=SLICES),
                    stage[st][NSUPER % 2].ap()[0:8, :].rearrange(
                        "s (t c) -> s t c", t=FCG),
                ).then_inc(sem[st, "osem0" if NSUPER % 2 == 0 else "osem1"], 16)

    return nc


def _chunk_starts():
    out = []
    for core in range(NCORES):
        sts = []
        for stm in range(STREAMS):
            sts.append(max((core * STREAMS + stm) * CHUNK - WARM, 0))
        out.append(sts)
    return out


def kernel(**inputs):
    from concourse.bass_utils import run_bass_kernel_spmd

    dt = np.float16 if FP16 else np.float32
    x = np.ascontiguousarray(
        np.asarray(inputs["x"], np.float32).reshape(T, B)).astype(dt)
    wblk, wfc = _prep_weights(
        np.asarray(inputs["w_ih"], np.float32), np.asarray(inputs["w_hh"], np.float32),
        np.asarray(inputs["b_ih"], np.float32), np.asarray(inputs["b_hh"], np.float32),
        np.asarray(inputs["w_fc"], np.float32), np.asarray(inputs["b_fc"], np.float32))

    nc = _build_program()
    starts = _chunk_starts()
    in_maps = []
    for core in range(NCORES):
        xc = np.zeros((STREAMS, NT, B), dt)
        for stm in range(STREAMS):
            g0 = starts[core][stm]
            xc[stm] = x[g0:g0 + NT]
        in_maps.append({"xc": xc, "wblk": wblk, "wfc": wfc})

    res = run_bass_kernel_spmd(nc, in_maps, core_ids=list(range(NCORES)))

    y = np.empty((T, B), np.float32)
    for core in range(NCORES):
        yc = res.results[core]["yc"]
        for stm in range(STREAMS):
            out0 = (core * STREAMS + stm) * CHUNK
            g0 = starts[core][stm]
            r0 = out0 - g0 + 1
            y[out0:out0 + CHUNK] = yc[stm, r0:r0 + CHUNK]
    return y.reshape(T, B, 1)

